# revision 3
# baseline (speedup 1.0000x reference)
"""Trainium2 Bass kernel for ForgetMult: h_t = f_t*x_t + (1-f_t)*h_{t-1}.

Full shapes: f, x [SEQ=1024, B=32, H=1024] fp32, hidden_init [32, 1024].
Output: stacked h over time, [1024, 32, 1024] fp32.

Strategy: the recurrence is independent per (b, h) lane. Shard B across the
8 cores (4 batches/core -> 4096 lanes/core). Host-side, repack each core's
inputs lane-major as [128 partitions, 32 lane-groups, 1024 time] so every
lane's full time series is contiguous in the SBUF free dimension. On device:
  a = 1 - f            (ScalarE activation, scale=-1 bias=1)
  b = f * x            (GpSimdE tensor multiply)
  h = scan(a, b, h0)   (VectorE tensor_tensor_scan: state = a*state + b)
one scan instruction per 128-lane group covers all 1024 timesteps. Output is
written back lane-major and un-packed on the host during the gather.
"""

import numpy as np

SEQ, B, H = 1024, 32, 1024
NCORES = 8
B_LOC = B // NCORES          # 4 batches per core
LGROUPS = B_LOC * H // 128   # 32 lane-groups of 128 lanes per core
GRP = 4                      # lane-groups per SBUF tile -> [128, 4, 1024] tiles
NTILES = LGROUPS // GRP


def _build_bass():
    import concourse.tile as tile
    from concourse import bacc, mybir

    f32 = mybir.dt.float32
    nc = bacc.Bacc("TRN2", target_bir_lowering=False, debug=False)
    f_d = nc.dram_tensor("f", [128, LGROUPS, SEQ], f32, kind="ExternalInput").ap()
    x_d = nc.dram_tensor("x", [128, LGROUPS, SEQ], f32, kind="ExternalInput").ap()
    h0_d = nc.dram_tensor("h0", [128, LGROUPS], f32, kind="ExternalInput").ap()
    o_d = nc.dram_tensor("out", [128, LGROUPS, SEQ], f32, kind="ExternalOutput").ap()

    with tile.TileContext(nc) as tc:
        with (
            tc.tile_pool(name="io", bufs=2) as io,
            tc.tile_pool(name="wk", bufs=2) as wk,
            tc.tile_pool(name="cst", bufs=1) as cst,
        ):
            h0_t = cst.tile([128, LGROUPS], f32)
            nc.sync.dma_start(h0_t[:], h0_d[:])
            for g in range(NTILES):
                sl = slice(g * GRP, (g + 1) * GRP)
                ft = io.tile([128, GRP, SEQ], f32, tag="f")
                nc.sync.dma_start(ft[:], f_d[:, sl, :])
                xt = io.tile([128, GRP, SEQ], f32, tag="x")
                nc.sync.dma_start(xt[:], x_d[:, sl, :])
                at = wk.tile([128, GRP, SEQ], f32, tag="a")
                nc.scalar.activation(
                    at[:], ft[:],
                    mybir.ActivationFunctionType.Identity,
                    bias=1.0, scale=-1.0,
                )
                bt = wk.tile([128, GRP, SEQ], f32, tag="b")
                nc.gpsimd.tensor_mul(bt[:], ft[:], xt[:])
                ht = io.tile([128, GRP, SEQ], f32, tag="h")
                for j in range(GRP):
                    lg = g * GRP + j
                    nc.vector.tensor_tensor_scan(
                        ht[:, j, :], at[:, j, :], bt[:, j, :],
                        h0_t[:, lg:lg + 1],
                        mybir.AluOpType.mult, mybir.AluOpType.add,
                    )
                nc.sync.dma_start(o_d[:, sl, :], ht[:])
    nc.compile()
    return nc


def _shard_inputs(f, x, hidden_init):
    # lane = b_loc*H + h; split lane -> (lg = b_loc*8 + h//128, p = h%128)
    # device layout per core: [p, lg, t]
    fr = (
        f.reshape(SEQ, NCORES, B_LOC, 8, 128)
        .transpose(1, 4, 2, 3, 0)
        .reshape(NCORES, 128, LGROUPS, SEQ)
    )
    xr = (
        x.reshape(SEQ, NCORES, B_LOC, 8, 128)
        .transpose(1, 4, 2, 3, 0)
        .reshape(NCORES, 128, LGROUPS, SEQ)
    )
    h0r = (
        hidden_init.reshape(NCORES, B_LOC, 8, 128)
        .transpose(0, 3, 1, 2)
        .reshape(NCORES, 128, LGROUPS)
    )
    return (
        np.ascontiguousarray(fr),
        np.ascontiguousarray(xr),
        np.ascontiguousarray(h0r),
    )


def _gather_output(outs):
    # outs: [NCORES, 128, LGROUPS, SEQ] -> [SEQ, B, H]
    return np.ascontiguousarray(
        outs.reshape(NCORES, 128, B_LOC, 8, SEQ)
        .transpose(4, 0, 2, 3, 1)
        .reshape(SEQ, B, H)
    )


def kernel(f, x, hidden_init):
    from concourse.bass_utils import run_bass_kernel_spmd

    f = np.asarray(f, dtype=np.float32)
    x = np.asarray(x, dtype=np.float32)
    hidden_init = np.asarray(hidden_init, dtype=np.float32)

    fr, xr, h0r = _shard_inputs(f, x, hidden_init)
    in_maps = [{"f": fr[k], "x": xr[k], "h0": h0r[k]} for k in range(NCORES)]

    nc = _build_bass()
    res = run_bass_kernel_spmd(nc, in_maps, list(range(NCORES)))
    outs = np.stack([res.results[k]["out"] for k in range(NCORES)])
    return _gather_output(outs)


# revision 4
# speedup vs baseline: 1.2587x; 1.2587x over previous
"""Trainium2 Bass kernel for ForgetMult: h_t = f_t*x_t + (1-f_t)*h_{t-1}.

Full shapes: f, x [SEQ=1024, B=32, H=1024] fp32, hidden_init [32, 1024].
Output: stacked h over time, [1024, 32, 1024] fp32.

Strategy: the recurrence is independent per (b, h) lane. Shard B across the
8 cores (4 batches/core -> 4096 lanes/core). Host-side, repack each core's
inputs lane-major as [128 partitions, 32 lane-groups, 1024 time] so every
lane's full time series is contiguous in the SBUF free dimension. On device:
  a = 1 - f            (ScalarE activation, scale=-1 bias=1)
  b = f * x            (GpSimdE tensor multiply)
  h = scan(a, b, h0)   (VectorE tensor_tensor_scan: state = a*state + b)
one scan instruction per 128-lane group covers all 1024 timesteps. Output is
written back lane-major and un-packed on the host during the gather.
"""

import numpy as np

SEQ, B, H = 1024, 32, 1024
NCORES = 8
B_LOC = B // NCORES          # 4 batches per core
LGROUPS = B_LOC * H // 128   # 32 lane-groups of 128 lanes per core
GRP = 4                      # lane-groups per SBUF tile -> [128, 4, 1024] tiles
NTILES = LGROUPS // GRP


def _build_bass():
    import concourse.tile as tile
    from concourse import bacc, mybir

    f32 = mybir.dt.float32
    nc = bacc.Bacc("TRN2", target_bir_lowering=False, debug=False)
    f_d = nc.dram_tensor("f", [128, LGROUPS, SEQ], f32, kind="ExternalInput").ap()
    x_d = nc.dram_tensor("x", [128, LGROUPS, SEQ], f32, kind="ExternalInput").ap()
    h0_d = nc.dram_tensor("h0", [128, LGROUPS], f32, kind="ExternalInput").ap()
    o_d = nc.dram_tensor("out", [128, LGROUPS, SEQ], f32, kind="ExternalOutput").ap()

    with tile.TileContext(nc) as tc:
        with (
            tc.tile_pool(name="io", bufs=3) as io,
            tc.tile_pool(name="cst", bufs=1) as cst,
        ):
            h0_t = cst.tile([128, LGROUPS], f32)
            nc.sync.dma_start(h0_t[:], h0_d[:])
            for g in range(NTILES):
                sl = slice(g * GRP, (g + 1) * GRP)
                ft = io.tile([128, GRP, SEQ], f32, tag="f")
                nc.sync.dma_start(ft[:], f_d[:, sl, :])
                xt = io.tile([128, GRP, SEQ], f32, tag="x")
                nc.sync.dma_start(xt[:], x_d[:, sl, :])
                # a = 1 - f on ScalarE (runs in parallel with the DVE mult)
                at = io.tile([128, GRP, SEQ], f32, tag="a")
                nc.scalar.activation(
                    at[:], ft[:],
                    mybir.ActivationFunctionType.Identity,
                    bias=1.0, scale=-1.0,
                )
                # b = f * x in place into xt (DVE; GpSimd shares the DVE SBUF
                # port and slows the scans, so keep it off the hot path)
                nc.vector.tensor_mul(xt[:], ft[:], xt[:])
                # h = scan(a, b) in place into at, one scan per lane-group
                for j in range(GRP):
                    lg = g * GRP + j
                    nc.vector.tensor_tensor_scan(
                        at[:, j, :], at[:, j, :], xt[:, j, :],
                        h0_t[:, lg:lg + 1],
                        mybir.AluOpType.mult, mybir.AluOpType.add,
                    )
                nc.sync.dma_start(o_d[:, sl, :], at[:])
    nc.compile()
    return nc


def _shard_inputs(f, x, hidden_init):
    # lane = b_loc*H + h; split lane -> (lg = b_loc*8 + h//128, p = h%128)
    # device layout per core: [p, lg, t]
    fr = (
        f.reshape(SEQ, NCORES, B_LOC, 8, 128)
        .transpose(1, 4, 2, 3, 0)
        .reshape(NCORES, 128, LGROUPS, SEQ)
    )
    xr = (
        x.reshape(SEQ, NCORES, B_LOC, 8, 128)
        .transpose(1, 4, 2, 3, 0)
        .reshape(NCORES, 128, LGROUPS, SEQ)
    )
    h0r = (
        hidden_init.reshape(NCORES, B_LOC, 8, 128)
        .transpose(0, 3, 1, 2)
        .reshape(NCORES, 128, LGROUPS)
    )
    return (
        np.ascontiguousarray(fr),
        np.ascontiguousarray(xr),
        np.ascontiguousarray(h0r),
    )


def _gather_output(outs):
    # outs: [NCORES, 128, LGROUPS, SEQ] -> [SEQ, B, H]
    return np.ascontiguousarray(
        outs.reshape(NCORES, 128, B_LOC, 8, SEQ)
        .transpose(4, 0, 2, 3, 1)
        .reshape(SEQ, B, H)
    )


def kernel(f, x, hidden_init):
    from concourse.bass_utils import run_bass_kernel_spmd

    f = np.asarray(f, dtype=np.float32)
    x = np.asarray(x, dtype=np.float32)
    hidden_init = np.asarray(hidden_init, dtype=np.float32)

    fr, xr, h0r = _shard_inputs(f, x, hidden_init)
    in_maps = [{"f": fr[k], "x": xr[k], "h0": h0r[k]} for k in range(NCORES)]

    nc = _build_bass()
    res = run_bass_kernel_spmd(nc, in_maps, list(range(NCORES)))
    outs = np.stack([res.results[k]["out"] for k in range(NCORES)])
    return _gather_output(outs)


# revision 6
# speedup vs baseline: 1.5425x; 1.2255x over previous
"""Trainium2 Bass kernel for ForgetMult: h_t = f_t*x_t + (1-f_t)*h_{t-1}.

Full shapes: f, x [SEQ=1024, B=32, H=1024] fp32, hidden_init [32, 1024].
Output: stacked h over time, [1024, 32, 1024] fp32.

Strategy: the recurrence is independent per (b, h) lane. Shard B across the
8 cores (4 batches/core -> 4096 lanes/core). Host-side, repack each core's
inputs lane-major as [128 partitions, 32 lane-groups, 1024 time] so every
lane's full time series is contiguous in the SBUF free dimension. On device:
  a = 1 - f            (ScalarE activation, scale=-1 bias=1)
  b = f * x            (GpSimdE tensor multiply)
  h = scan(a, b, h0)   (VectorE tensor_tensor_scan: state = a*state + b)
one scan instruction per 128-lane group covers all 1024 timesteps. Output is
written back lane-major and un-packed on the host during the gather.
"""

import numpy as np

SEQ, B, H = 1024, 32, 1024
NCORES = 8
B_LOC = B // NCORES          # 4 batches per core
LGROUPS = B_LOC * H // 128   # 32 lane-groups of 128 lanes per core
GRP = 4                      # lane-groups per SBUF tile -> [128, 4, 1024] tiles
NTILES = LGROUPS // GRP


def _build_bass():
    import concourse.tile as tile
    from concourse import bacc, mybir

    f32 = mybir.dt.float32
    nc = bacc.Bacc("TRN2", target_bir_lowering=False, debug=False)
    f_d = nc.dram_tensor("f", [128, LGROUPS, SEQ], f32, kind="ExternalInput").ap()
    x_d = nc.dram_tensor("x", [128, LGROUPS, SEQ], f32, kind="ExternalInput").ap()
    h0_d = nc.dram_tensor("h0", [128, LGROUPS], f32, kind="ExternalInput").ap()
    o_d = nc.dram_tensor("out", [128, LGROUPS, SEQ], f32, kind="ExternalOutput").ap()

    with tile.TileContext(nc) as tc:
        with (
            tc.tile_pool(name="io", bufs=3) as io,
            tc.tile_pool(name="cst", bufs=1) as cst,
        ):
            h0_t = cst.tile([128, LGROUPS], f32)
            nc.sync.dma_start(h0_t[:], h0_d[:])
            half = GRP // 2
            for g in range(NTILES):
                sl = slice(g * GRP, (g + 1) * GRP)
                slo = slice(g * GRP, g * GRP + half)
                shi = slice(g * GRP + half, (g + 1) * GRP)
                # split every transfer across the two HWDGE rings (SP + ACT)
                # so loads/stores interleave instead of serializing on one
                ft = io.tile([128, GRP, SEQ], f32, tag="f")
                nc.sync.dma_start(ft[:, 0:half, :], f_d[:, slo, :])
                nc.scalar.dma_start(ft[:, half:GRP, :], f_d[:, shi, :])
                xt = io.tile([128, GRP, SEQ], f32, tag="x")
                nc.sync.dma_start(xt[:, 0:half, :], x_d[:, slo, :])
                nc.scalar.dma_start(xt[:, half:GRP, :], x_d[:, shi, :])
                # a = 1 - f on ScalarE (runs in parallel with the DVE mult)
                at = io.tile([128, GRP, SEQ], f32, tag="a")
                nc.scalar.activation(
                    at[:], ft[:],
                    mybir.ActivationFunctionType.Identity,
                    bias=1.0, scale=-1.0,
                )
                # b = f * x in place into xt (DVE; GpSimd shares the DVE SBUF
                # port and slows the scans, so keep it off the hot path)
                nc.vector.tensor_mul(xt[:], ft[:], xt[:])
                # h = scan(a, b) in place into at, one scan per lane-group
                for j in range(GRP):
                    lg = g * GRP + j
                    nc.vector.tensor_tensor_scan(
                        at[:, j, :], at[:, j, :], xt[:, j, :],
                        h0_t[:, lg:lg + 1],
                        mybir.AluOpType.mult, mybir.AluOpType.add,
                    )
                nc.sync.dma_start(o_d[:, slo, :], at[:, 0:half, :])
                nc.scalar.dma_start(o_d[:, shi, :], at[:, half:GRP, :])
    nc.compile()
    return nc


def _shard_inputs(f, x, hidden_init):
    # lane = b_loc*H + h; split lane -> (lg = b_loc*8 + h//128, p = h%128)
    # device layout per core: [p, lg, t]
    fr = (
        f.reshape(SEQ, NCORES, B_LOC, 8, 128)
        .transpose(1, 4, 2, 3, 0)
        .reshape(NCORES, 128, LGROUPS, SEQ)
    )
    xr = (
        x.reshape(SEQ, NCORES, B_LOC, 8, 128)
        .transpose(1, 4, 2, 3, 0)
        .reshape(NCORES, 128, LGROUPS, SEQ)
    )
    h0r = (
        hidden_init.reshape(NCORES, B_LOC, 8, 128)
        .transpose(0, 3, 1, 2)
        .reshape(NCORES, 128, LGROUPS)
    )
    return (
        np.ascontiguousarray(fr),
        np.ascontiguousarray(xr),
        np.ascontiguousarray(h0r),
    )


def _gather_output(outs):
    # outs: [NCORES, 128, LGROUPS, SEQ] -> [SEQ, B, H]
    return np.ascontiguousarray(
        outs.reshape(NCORES, 128, B_LOC, 8, SEQ)
        .transpose(4, 0, 2, 3, 1)
        .reshape(SEQ, B, H)
    )


def kernel(f, x, hidden_init):
    from concourse.bass_utils import run_bass_kernel_spmd

    f = np.asarray(f, dtype=np.float32)
    x = np.asarray(x, dtype=np.float32)
    hidden_init = np.asarray(hidden_init, dtype=np.float32)

    fr, xr, h0r = _shard_inputs(f, x, hidden_init)
    in_maps = [{"f": fr[k], "x": xr[k], "h0": h0r[k]} for k in range(NCORES)]

    nc = _build_bass()
    res = run_bass_kernel_spmd(nc, in_maps, list(range(NCORES)))
    outs = np.stack([res.results[k]["out"] for k in range(NCORES)])
    return _gather_output(outs)
